# revision 26
# baseline (speedup 1.0000x reference)
"""Associative-embedding loss on 8 Trainium2 NeuronCores.

Data-parallel over batch N=32: each of the 8 cores handles 4 images;
per-image losses are independent, the final means are taken on the host.

Gather: the per-image flat tag array (1114112 f32) is viewed as 17408
rows of 64 f32 (256B).  One SWDGE dma_gather per image (512 int16 row
indices, 994ns fixed Q7 cost each) fetches the row containing each
needed element — vs 17 per-element indirect DMAs (~1us each) in the
naive approach.  DMAGatherAnt needs the 'mlp' Q7 library; the reload
pseudo-instruction is hand-assembled because only Bacc's codegen path
normally emits its bytes.  Row indices (loc//64) and a {0,1} bf16
selection one-hot (loc%64) are precomputed on the host (pure address
arithmetic; all tag data is only ever touched on-device).

Cell layout: image cell i = person*17 + joint -> gather dest partition
i%128, slot i//128, so 510 real cells pack into exactly 512 = 4 slots.

Per chunk (image), pipelined against the remaining gathers:
rows*onehot -> X-reduce -> A[p,s]; gsv=A*vis, gsv2=gsv*A; 4 accumulating
K=128 matmuls against constant per-slot indicators reduce cells to
per-person (sum v*g, sum v*g^2).  pull uses the variance identity
(S2/c - mean^2).  push: one K=128 matmul against a block-diagonal
constant replicates smm = mean + BIG*(1-valid) to every partition;
ACT Derivative_Erf(x) = (2/sqrt(pi))exp(-x^2) with bias=-mean computes
all pairwise exp(-(m_j-m_p)^2) terms in a single instruction with a
fused row-sum accumulator (invalid persons pushed to ~BIG kill their
columns; the diagonal's derf(0) is removed via the per-image tag count).
Per-image segment sums via PE matmuls against (validity-gated) segment
indicators.  All visibility-derived scalars (cnt, 1/cnt, valid, pair
counts) are computed while the gathers are still in flight.
"""

import numpy as np
import ml_dtypes
from contextlib import ExitStack

import concourse.bass as bass
import concourse.tile as tile
from concourse import library_config, mybir
from concourse.bass_utils import run_bass_kernel_spmd

# Problem constants (hardcoded per contract).
N, K, H, W, M = 32, 17, 256, 256, 30
NCORES = 8
NLOC = N // NCORES          # images per core
KHW = K * H * W             # 1114112 flat tag elements per image
MP = 32                     # padded persons per image
P = 128                     # SBUF partitions
BIG = 30.0                  # exp(-BIG) ~ 9e-14: kills masked columns

ROW = 64                    # gather row width (f32 elems, 256B)
NROWS = KHW // ROW          # 17408 rows per image (< int16 range)
SLOTS = 4                   # gather slots per partition per image
NI = SLOTS * P              # 512 idxs per per-image gather
CELLS = NLOC * SLOTS        # 16 scattered cells per partition

# blob column offsets
C_VIS = 0                   # [128, 20] raw vis values, scattered layout
C_VISP = C_VIS + CELLS      # [128, 17] raw vis values, person layout
C_PERS = C_VISP + K         # [128, 30] PERS30[p, j] = (p%32 == j)
C_IND = C_PERS + M          # [128, 4*32] IND4[p, s*32+m] = (cell (p,s) of person m)
C_SEG = C_IND + SLOTS * MP  # [128, 4]  SEG[p, i] = (p//32==i and p%32<30)
C_IMG = C_SEG + NLOC        # [128, 128] IMGBLK[q, p] = (q//32 == p//32)
BLOB = C_IMG + P            # 327

# push math via Derivative_Erf(x) = (2/sqrt(pi)) * exp(-x^2):
#   arg[p,j] = mean_j + BIGK*(1-valid_j) - mean_p
# valid j != p: exp(-(mean_j-mean_p)^2); invalid j: exp(-(~BIGK)^2) ~ 0;
# diagonal (valid p): derf(0) = 2/sqrt(pi), removed per image via
# psF1 - (2/sqrt(pi))*nt before scaling by sqrt(pi)/2.
SQPI_4 = 0.44311346272637900682   # sqrt(pi)/4

f32 = mybir.dt.float32
bf16 = mybir.dt.bfloat16
i16 = mybir.dt.int16
Alu = mybir.AluOpType
Act = mybir.ActivationFunctionType
AxX = mybir.AxisListType.X


def build_nc() -> bass.Bass:
    nc = bass.Bass()
    tags_d = nc.declare_dram_parameter("tags", [NLOC, NROWS, ROW], f32,
                                       isOutput=False)
    idx_d = nc.declare_dram_parameter("idx", [P, NLOC * NI // 16], i16,
                                      isOutput=False)
    blob_d = nc.declare_dram_parameter("blob", [P, BLOB], f32, isOutput=False)
    oh_d = nc.declare_dram_parameter("oh", [P, CELLS * ROW], bf16,
                                     isOutput=False)
    out_d = nc.declare_dram_parameter("out", [NLOC, 2], f32, isOutput=True)

    with tile.TileContext(nc) as tc:
        with ExitStack() as ctx:
            _body(ctx, tc, nc, tags_d, idx_d[:], blob_d[:], oh_d[:], out_d[:])
    _split_multi_waits(nc, max_waits=1)
    return nc


def _split_multi_waits(nc, max_waits=1):
    """Walrus codegen rejects instructions with too many sync-wait commands
    ("Too many sync wait commands", CoreV3GenImpl::setupSyncWait). Tile's
    kernel-tail drain waits on every live semaphore. Split the excess waits
    onto same-engine nops inserted immediately before the offending
    instruction — identical semantics, one wait per instruction."""
    import bass_rust
    fn = nc.m.functions[0]
    for bb in fn.blocks:
        changed = True
        while changed:
            changed = False
            for inst in list(bb.instructions):
                si = inst.sync_info
                if si is None or not si.on_wait or len(si.on_wait) <= max_waits:
                    continue
                waits = list(si.on_wait)
                keep, rest = waits[:max_waits], waits[max_waits:]
                nops = []
                for i in range(0, len(rest), max_waits):
                    nop_inst = nc.engines[inst.engine].nop().ins
                    nop_inst.sync_info = bass_rust.SyncInfo(
                        on_wait=rest[i:i + max_waits], on_update=[])
                    nops.append(nop_inst)
                inst.sync_info = bass_rust.SyncInfo(
                    on_wait=keep, on_update=list(si.on_update))
                for b2 in fn.blocks:
                    lst = b2.instructions
                    for i in range(len(lst) - 1, -1, -1):
                        if any(lst[i].name == n.name for n in nops):
                            del lst[i]
                idx = next(i for i, x in enumerate(bb.instructions)
                           if x.name == inst.name)
                for j, n in enumerate(nops):
                    bb.instructions.insert(idx + j, n)
                changed = True
                break


def _body(ctx, tc, nc, tags_d, idx, blob, oh, out):
    pool = ctx.enter_context(tc.tile_pool(name="main", bufs=1))
    psum = ctx.enter_context(tc.tile_pool(name="psum", bufs=1, space="PSUM"))

    # DMAGatherAnt lives in the 'mlp' Q7 library; load it before any gather.
    # The plain-Bass path never assembles the pseudo's instruction bytes
    # (only Bacc's codegen does), which walrus rejects as "ISA wrong length".
    # Assemble the NEURON_ISA_TPB_PSEUDO_LIBRARY_RELOAD_INDEX_STRUCT bytes
    # ourselves so walrus emits a runtime-interpreted PSEUDO_INST.
    import concourse.bass_isa as bass_isa
    reload_inst = nc.gpsimd.load_library(library_config.mlp)
    pseudo_op = nc.isa.get_enum("NEURON_ISA_TPB_PSEUDO_OPCODE")
    instr_bytes, _ = bass_isa.isa_struct(
        nc.isa, nc.isa.Opcode.NEURON_ISA_TPB_OPCODE_PSEUDO_INST,
        {"pseudo_opcode":
         pseudo_op.NEURON_ISA_TPB_PSEUDO_OPCODE_PSEUDO_LIBRARY_RELOAD_INDEX.value,
         "lib_index": library_config.mlp.index},
        struct_name="NEURON_ISA_TPB_PSEUDO_LIBRARY_RELOAD_INDEX_STRUCT")
    reload_inst.ins.instr = instr_bytes

    # ---- input DMAs ---------------------------------------------------------
    # chunk-0 idxs ride alone so gather 0 can start ~200ns earlier; the
    # remaining chunks' idxs arrive well before their gathers issue.
    NIC = NI // 16
    sb_idx = pool.tile([P, NLOC * NIC], i16)
    nc.sync.dma_start(out=sb_idx[:, 0:NIC], in_=idx[:, 0:NIC])
    nc.sync.dma_start(out=sb_idx[:, NIC:], in_=idx[:, NIC:])
    sb_blob = pool.tile([P, BLOB], f32)
    nc.sync.dma_start(out=sb_blob, in_=blob)
    sb_oh = pool.tile([P, CELLS, ROW], bf16)
    nc.sync.dma_start(out=sb_oh, in_=oh)

    vis_scat = sb_blob[:, C_VIS:C_VIS + CELLS]
    visp = sb_blob[:, C_VISP:C_VISP + K]
    pers30 = sb_blob[:, C_PERS:C_PERS + M]
    ind4 = sb_blob[:, C_IND:C_IND + SLOTS * MP]
    seg = sb_blob[:, C_SEG:C_SEG + NLOC]
    imgblk = sb_blob[:, C_IMG:C_IMG + P]

    # ---- per-image gathers (Pool SWDGE), issued ASAP ------------------------
    rows = []
    for c in range(NLOC):
        rows_c = pool.tile([P, SLOTS, ROW], f32, tag=f"rows{c}")
        nc.gpsimd.dma_gather(
            out_ap=rows_c[:],
            in_ap=tags_d[c],
            idxs_ap=sb_idx[:, c * (NI // 16):(c + 1) * (NI // 16)],
            num_idxs=NI,
            num_idxs_reg=NI,
            elem_size=ROW,
        )
        rows.append(rows_c)

    # ---- visibility-derived scalars (overlap the gathers) -------------------
    visf = pool.tile([P, CELLS], f32)      # scattered-layout 0/1 mask
    nc.vector.tensor_scalar(out=visf, in0=vis_scat, scalar1=0.0, scalar2=None,
                            op0=Alu.is_gt)
    visfp = pool.tile([P, K], f32)         # person-layout 0/1 mask
    nc.vector.tensor_scalar(out=visfp, in0=visp, scalar1=0.0, scalar2=None,
                            op0=Alu.is_gt)
    cnt = pool.tile([P, 1], f32)
    nc.vector.reduce_sum(out=cnt, in_=visfp, axis=AxX)
    valid = pool.tile([P, 1], f32)
    nc.vector.tensor_scalar(out=valid, in0=cnt, scalar1=1.0, scalar2=None,
                            op0=Alu.min)
    safecnt = pool.tile([P, 1], f32)
    nc.vector.tensor_scalar(out=safecnt, in0=cnt, scalar1=1.0, scalar2=None,
                            op0=Alu.max)
    rc = pool.tile([P, 1], f32)
    nc.vector.reciprocal(out=rc, in_=safecnt)
    sm = pool.tile([P, 1], f32)            # BIGK*(1-valid)
    nc.vector.tensor_scalar(out=sm, in0=valid, scalar1=-BIG, scalar2=BIG,
                            op0=Alu.mult, op1=Alu.add)
    segv = pool.tile([P, NLOC], f32)       # SEG gated by person validity
    nc.vector.tensor_scalar(out=segv, in0=seg, scalar1=valid, scalar2=None,
                            op0=Alu.mult)

    # per-image tag counts + push denominators (all early, off critical path)
    psNT = psum.tile([NLOC, 1], f32, tag="psNT")
    nc.tensor.matmul(out=psNT[:], lhsT=seg, rhs=valid, start=True, stop=True)
    nt = pool.tile([NLOC, 1], f32)
    nc.vector.tensor_copy(out=nt, in_=psNT)
    sant = pool.tile([NLOC, 1], f32)
    nc.vector.tensor_scalar(out=sant, in0=nt, scalar1=1.0, scalar2=None,
                            op0=Alu.max)
    rnt = pool.tile([NLOC, 1], f32)
    nc.vector.reciprocal(out=rnt, in_=sant)
    npr = pool.tile([NLOC, 1], f32)
    nc.vector.scalar_tensor_tensor(out=npr, in0=nt, scalar=-1.0, in1=nt,
                                   op0=Alu.add, op1=Alu.mult)  # (nt-1)*nt
    gate = pool.tile([NLOC, 1], f32)
    nc.vector.tensor_scalar(out=gate, in0=npr, scalar1=0.0, scalar2=None,
                            op0=Alu.is_gt)
    sanp = pool.tile([NLOC, 1], f32)
    nc.vector.tensor_scalar(out=sanp, in0=npr, scalar1=1.0, scalar2=None,
                            op0=Alu.max)
    rnp = pool.tile([NLOC, 1], f32)
    nc.vector.reciprocal(out=rnp, in_=sanp)
    ga = pool.tile([NLOC, 1], f32)         # sqrt(pi)/4 * gate * rnp
    nc.vector.scalar_tensor_tensor(out=ga, in0=gate, scalar=SQPI_4, in1=rnp,
                                   op0=Alu.mult, op1=Alu.mult)
    gb0 = pool.tile([NLOC, 1], f32)
    nc.vector.scalar_tensor_tensor(out=gb0, in0=rnp, scalar=0.5, in1=nt,
                                   op0=Alu.mult, op1=Alu.mult)
    gb = pool.tile([NLOC, 1], f32)         # 0.5 * nt * gate * rnp
    nc.vector.tensor_tensor(out=gb, in0=gb0, in1=gate, op=Alu.mult)

    # ---- per-chunk selection + cell stats + person-stats matmul -------------
    A = pool.tile([P, CELLS], f32)
    GV = pool.tile([P, 2, CELLS], f32)
    psS = psum.tile([P, 2], f32, tag="psS")

    for c in range(NLOC):
        cs = slice(c * SLOTS, (c + 1) * SLOTS)
        sel_c = pool.tile([P, SLOTS, ROW], f32, tag=f"sel{c}")
        nc.vector.tensor_tensor(out=sel_c, in0=rows[c][:],
                                in1=sb_oh[:, cs, :], op=Alu.mult)
        nc.vector.reduce_sum(out=A[:, cs], in_=sel_c[:], axis=AxX)
        nc.vector.tensor_tensor(out=GV[:, 0, cs], in0=A[:, cs],
                                in1=visf[:, cs], op=Alu.mult)
        nc.vector.tensor_tensor(out=GV[:, 1, cs], in0=GV[:, 0, cs],
                                in1=A[:, cs], op=Alu.mult)
        for s in range(SLOTS):
            nc.tensor.matmul(out=psS[c * MP:(c + 1) * MP, :],
                             lhsT=ind4[:, s * MP:(s + 1) * MP],
                             rhs=GV[:, :, c * SLOTS + s],
                             start=(s == 0), stop=(s == SLOTS - 1),
                             tile_position=(0, c * MP))

    # ---- per-person mean / push arg / pull ----------------------------------
    smm = pool.tile([P, 1], f32)           # mean + BIGK*(1-valid)
    nc.vector.scalar_tensor_tensor(out=smm, in0=psS[:, 0:1], scalar=rc,
                                   in1=sm, op0=Alu.mult, op1=Alu.add)
    mrhs = pool.tile([P, M], f32)
    nc.vector.tensor_scalar(out=mrhs, in0=pers30, scalar1=smm, scalar2=None,
                            op0=Alu.mult)
    psRep = psum.tile([P, M], f32, tag="psRep")
    nc.tensor.matmul(out=psRep[:], lhsT=imgblk, rhs=mrhs[:],
                     start=True, stop=True)
    negmean = pool.tile([P, 1], f32)
    nc.vector.tensor_scalar(out=negmean, in0=psS[:, 0:1], scalar1=rc,
                            scalar2=-1.0, op0=Alu.mult, op1=Alu.mult)
    mean2 = pool.tile([P, 1], f32)
    nc.vector.tensor_tensor(out=mean2, in0=negmean, in1=negmean, op=Alu.mult)
    p1 = pool.tile([P, 1], f32)
    nc.vector.scalar_tensor_tensor(out=p1, in0=psS[:, 1:2], scalar=rc,
                                   in1=mean2, op0=Alu.mult, op1=Alu.subtract)
    pullred = pool.tile([P, 1], f32)
    nc.vector.tensor_tensor(out=pullred, in0=p1, in1=valid, op=Alu.mult)

    # derf(arg) = (2/sqrt(pi)) exp(-arg^2); row sum in the ACT accumulator
    pe = pool.tile([P, M], f32)
    rowsumv = pool.tile([P, 1], f32)
    nc.scalar.activation(out=pe, in_=psRep[:], func=Act.Derivative_Erf,
                         bias=negmean, accum_out=rowsumv)

    # ---- per-image segment sums + finals ------------------------------------
    psF = psum.tile([NLOC, 2], f32, tag="psF")
    nc.tensor.matmul(out=psF[:, 0:1], lhsT=seg, rhs=pullred[:],
                     start=True, stop=True)
    nc.tensor.matmul(out=psF[:, 1:2], lhsT=segv, rhs=rowsumv[:],
                     start=True, stop=True)
    f42 = pool.tile([NLOC, 2], f32)
    nc.vector.tensor_scalar(out=f42[:, 0:1], in0=psF[:, 0:1], scalar1=rnt,
                            scalar2=None, op0=Alu.mult)
    nc.vector.scalar_tensor_tensor(out=f42[:, 1:2], in0=psF[:, 1:2],
                                   scalar=ga, in1=gb, op0=Alu.mult,
                                   op1=Alu.subtract)
    nc.sync.dma_start(out=out, in_=f42)


# ---------------------------------------------------------------------------
# host side
# ---------------------------------------------------------------------------

def _build_consts():
    p = np.arange(P)
    j30 = np.arange(M)
    pers30 = ((p[:, None] % MP) == j30[None, :]).astype(np.float32)
    # IND4[p, s*32+m] = 1 iff gather cell i = s*128+p is a real cell of
    # person m (column-major packing: cell i == m*K + k for i < M*K)
    i_grid = np.arange(SLOTS)[:, None] * P + p[None, :]      # [4, 128]
    ind4 = np.zeros((P, SLOTS, MP), dtype=np.float32)
    for s in range(SLOTS):
        mcell = i_grid[s] // K
        real = i_grid[s] < M * K
        ind4[real, s, mcell[real]] = 1.0
    seg = (((p[:, None] // MP) == np.arange(NLOC)[None, :])
           & ((p[:, None] % MP) < M)).astype(np.float32)
    imgblk = ((p[:, None] // MP) == (p[None, :] // MP)).astype(np.float32)
    return np.concatenate([pers30, ind4.reshape(P, SLOTS * MP), seg, imgblk],
                          axis=1).astype(np.float32)  # [128, 290]


_CONSTS = _build_consts()

# cell mapping: joint k of person m -> cell i = m*K + k; partition i%128,
# slot i//128 (matches the gather's idx->dest wrap)
_MM, _KK = np.meshgrid(np.arange(M), np.arange(K), indexing="ij")
_CI = (_MM * K + _KK).ravel()                       # [510]
_CELL_P = _CI % P
_CELL_S = _CI // P
_IDX_I = _CI                                        # gather idx position i


def make_in_maps(tags: np.ndarray, joints: np.ndarray):
    tags = np.ascontiguousarray(np.asarray(tags, dtype=np.float32))
    jt = np.asarray(joints)
    loc = np.clip(jt[..., 0], 0, KHW - 1).astype(np.int64)   # [N, M, K]
    visraw = jt[..., 1].astype(np.float32)                   # [N, M, K]
    row = (loc // ROW).astype(np.int16)                      # [N, M, K]
    sub = (loc % ROW).astype(np.int64)                       # [N, M, K]

    # gather idx arrays [N, 640] -> SBUF wrap [N, 128, 40]
    idx_all = np.zeros((N, NI), dtype=np.int16)
    idx_all[:, _IDX_I] = row.reshape(N, M * K)
    idx_sb = np.tile(idx_all.reshape(N, NI // 16, 16).transpose(0, 2, 1),
                     (1, 8, 1))                              # [N, 128, 40]

    # selection one-hot [N, 128, SLOTS*ROW]
    onehot = np.zeros((N, P, SLOTS * ROW), dtype=np.float32)
    cellflat = (_CELL_S * ROW)[None, :] + sub.reshape(N, M * K)  # [N, 510]
    onehot[np.arange(N)[:, None], _CELL_P[None, :], cellflat] = 1.0

    # scattered vis [N, 128, SLOTS]
    vis_scat = np.zeros((N, P, SLOTS), dtype=np.float32)
    vis_scat[np.arange(N)[:, None], _CELL_P[None, :],
             _CELL_S[None, :]] = visraw.reshape(N, M * K)

    # person-layout vis [N, 32, K]
    visp = np.zeros((N, MP, K), dtype=np.float32)
    visp[:, :M, :] = visraw

    in_maps = []
    for c in range(NCORES):
        sl = slice(c * NLOC, (c + 1) * NLOC)
        blob = np.concatenate([
            vis_scat[sl].transpose(1, 0, 2).reshape(P, CELLS),
            visp[sl].reshape(P, K),
            _CONSTS,
        ], axis=1)
        in_maps.append({
            "tags": tags[sl].reshape(NLOC, NROWS, ROW),
            "idx": np.ascontiguousarray(
                np.concatenate(idx_sb[sl], axis=1)),     # [128, 160]
            "blob": np.ascontiguousarray(blob),
            "oh": np.ascontiguousarray(
                onehot[sl].transpose(1, 0, 2).reshape(P, CELLS * ROW)
                .astype(ml_dtypes.bfloat16)),
        })
    return in_maps


_NC_CACHE = None


def _get_nc():
    global _NC_CACHE
    if _NC_CACHE is None:
        _NC_CACHE = build_nc()
    return _NC_CACHE


def kernel(tags: np.ndarray, joints: np.ndarray, _bench_results=None):
    nc = _get_nc()
    in_maps = make_in_maps(tags, joints)
    res = run_bass_kernel_spmd(nc, in_maps, core_ids=list(range(NCORES)))
    if _bench_results is not None:
        _bench_results.append(res)
    per_image = np.concatenate([r["out"] for r in res.results], axis=0)
    pull_loss = np.float32(per_image[:, 0].mean(dtype=np.float64))
    push_loss = np.float32(per_image[:, 1].mean(dtype=np.float64))
    return pull_loss, push_loss


# revision 31
# speedup vs baseline: 1.0047x; 1.0047x over previous
"""Associative-embedding loss on 8 Trainium2 NeuronCores.

Data-parallel over batch N=32: each of the 8 cores handles 4 images;
per-image losses are independent, the final means are taken on the host.

Gather: the per-image flat tag array (1114112 f32) is viewed as 17408
rows of 64 f32 (256B).  One SWDGE dma_gather per image (512 int16 row
indices, 994ns fixed Q7 cost each) fetches the row containing each
needed element — vs 17 per-element indirect DMAs (~1us each) in the
naive approach.  DMAGatherAnt needs the 'mlp' Q7 library; the reload
pseudo-instruction is hand-assembled because only Bacc's codegen path
normally emits its bytes.  Row indices (loc//64) and a {0,1} bf16
selection one-hot (loc%64) are precomputed on the host (pure address
arithmetic; all tag data is only ever touched on-device).

Cell layout: image cell i = person*17 + joint -> gather dest partition
i%128, slot i//128, so 510 real cells pack into exactly 512 = 4 slots.

Per chunk (image), pipelined against the remaining gathers:
rows*onehot -> X-reduce -> A[p,s]; gsv=A*vis, gsv2=gsv*A; 4 accumulating
K=128 matmuls against constant per-slot indicators reduce cells to
per-person (sum v*g, sum v*g^2).  pull uses the variance identity
(S2/c - mean^2).  push: one K=128 matmul against a block-diagonal
constant replicates smm = mean + BIG*(1-valid) to every partition;
ACT Derivative_Erf(x) = (2/sqrt(pi))exp(-x^2) with bias=-mean computes
all pairwise exp(-(m_j-m_p)^2) terms in a single instruction with a
fused row-sum accumulator (invalid persons pushed to ~BIG kill their
columns; the diagonal's derf(0) is removed via the per-image tag count).
Per-image segment sums via PE matmuls against (validity-gated) segment
indicators.  All visibility-derived scalars (cnt, 1/cnt, valid, pair
counts) are computed while the gathers are still in flight.
"""

import numpy as np
import ml_dtypes
from contextlib import ExitStack

import concourse.bass as bass
import concourse.tile as tile
from concourse import library_config, mybir
from concourse.bass_utils import run_bass_kernel_spmd

# Problem constants (hardcoded per contract).
N, K, H, W, M = 32, 17, 256, 256, 30
NCORES = 8
NLOC = N // NCORES          # images per core
KHW = K * H * W             # 1114112 flat tag elements per image
MP = 32                     # padded persons per image
P = 128                     # SBUF partitions
BIG = 30.0                  # exp(-BIG) ~ 9e-14: kills masked columns

ROW = 64                    # gather row width (f32 elems, 256B)
NROWS = KHW // ROW          # 17408 rows per image (< int16 range)
SLOTS = 4                   # gather slots per partition per image
NI = SLOTS * P              # 512 idxs per per-image gather
CELLS = NLOC * SLOTS        # 16 scattered cells per partition

# blob column offsets
C_VISP = 0                  # [128, 17] raw vis values, person layout
C_PERS = C_VISP + K         # [128, 30] PERS30[p, j] = (p%32 == j)
C_IND = C_PERS + M          # [128, 4*32] IND4[p, s*32+m] = (cell (p,s) of person m)
C_SEG = C_IND + SLOTS * MP  # [128, 4]  SEG[p, i] = (p//32==i and p%32<30)
C_IMG = C_SEG + NLOC        # [128, 128] IMGBLK[q, p] = (q//32 == p//32)
BLOB = C_IMG + P            # 327

# push math via Derivative_Erf(x) = (2/sqrt(pi)) * exp(-x^2):
#   arg[p,j] = mean_j + BIGK*(1-valid_j) - mean_p
# valid j != p: exp(-(mean_j-mean_p)^2); invalid j: exp(-(~BIGK)^2) ~ 0;
# diagonal (valid p): derf(0) = 2/sqrt(pi), removed per image via
# psF1 - (2/sqrt(pi))*nt before scaling by sqrt(pi)/2.
SQPI_4 = 0.44311346272637900682   # sqrt(pi)/4

f32 = mybir.dt.float32
bf16 = mybir.dt.bfloat16
i16 = mybir.dt.int16
Alu = mybir.AluOpType
Act = mybir.ActivationFunctionType
AxX = mybir.AxisListType.X


def build_nc() -> bass.Bass:
    nc = bass.Bass()
    tags_d = nc.declare_dram_parameter("tags", [NLOC, NROWS, ROW], f32,
                                       isOutput=False)
    idx_d = nc.declare_dram_parameter("idx", [P, NLOC * NI // 16], i16,
                                      isOutput=False)
    blob_d = nc.declare_dram_parameter("blob", [P, BLOB], f32, isOutput=False)
    oh_d = nc.declare_dram_parameter("oh", [P, CELLS * ROW], bf16,
                                     isOutput=False)
    out_d = nc.declare_dram_parameter("out", [NLOC, 2], f32, isOutput=True)

    with tile.TileContext(nc) as tc:
        with ExitStack() as ctx:
            _body(ctx, tc, nc, tags_d, idx_d[:], blob_d[:], oh_d[:], out_d[:])
    _split_multi_waits(nc, max_waits=1)
    return nc


def _split_multi_waits(nc, max_waits=1):
    """Walrus codegen rejects instructions with too many sync-wait commands
    ("Too many sync wait commands", CoreV3GenImpl::setupSyncWait). Tile's
    kernel-tail drain waits on every live semaphore. Split the excess waits
    onto same-engine nops inserted immediately before the offending
    instruction — identical semantics, one wait per instruction."""
    import bass_rust
    fn = nc.m.functions[0]
    for bb in fn.blocks:
        changed = True
        while changed:
            changed = False
            for inst in list(bb.instructions):
                si = inst.sync_info
                if si is None or not si.on_wait or len(si.on_wait) <= max_waits:
                    continue
                waits = list(si.on_wait)
                keep, rest = waits[:max_waits], waits[max_waits:]
                nops = []
                for i in range(0, len(rest), max_waits):
                    nop_inst = nc.engines[inst.engine].nop().ins
                    nop_inst.sync_info = bass_rust.SyncInfo(
                        on_wait=rest[i:i + max_waits], on_update=[])
                    nops.append(nop_inst)
                inst.sync_info = bass_rust.SyncInfo(
                    on_wait=keep, on_update=list(si.on_update))
                for b2 in fn.blocks:
                    lst = b2.instructions
                    for i in range(len(lst) - 1, -1, -1):
                        if any(lst[i].name == n.name for n in nops):
                            del lst[i]
                idx = next(i for i, x in enumerate(bb.instructions)
                           if x.name == inst.name)
                for j, n in enumerate(nops):
                    bb.instructions.insert(idx + j, n)
                changed = True
                break


def _body(ctx, tc, nc, tags_d, idx, blob, oh, out):
    pool = ctx.enter_context(tc.tile_pool(name="main", bufs=1))
    psum = ctx.enter_context(tc.tile_pool(name="psum", bufs=1, space="PSUM"))

    # DMAGatherAnt lives in the 'mlp' Q7 library; load it before any gather.
    # The plain-Bass path never assembles the pseudo's instruction bytes
    # (only Bacc's codegen does), which walrus rejects as "ISA wrong length".
    # Assemble the NEURON_ISA_TPB_PSEUDO_LIBRARY_RELOAD_INDEX_STRUCT bytes
    # ourselves so walrus emits a runtime-interpreted PSEUDO_INST.
    import concourse.bass_isa as bass_isa
    reload_inst = nc.gpsimd.load_library(library_config.mlp)
    pseudo_op = nc.isa.get_enum("NEURON_ISA_TPB_PSEUDO_OPCODE")
    instr_bytes, _ = bass_isa.isa_struct(
        nc.isa, nc.isa.Opcode.NEURON_ISA_TPB_OPCODE_PSEUDO_INST,
        {"pseudo_opcode":
         pseudo_op.NEURON_ISA_TPB_PSEUDO_OPCODE_PSEUDO_LIBRARY_RELOAD_INDEX.value,
         "lib_index": library_config.mlp.index},
        struct_name="NEURON_ISA_TPB_PSEUDO_LIBRARY_RELOAD_INDEX_STRUCT")
    reload_inst.ins.instr = instr_bytes

    # ---- input DMAs ---------------------------------------------------------
    # chunk-0 idxs ride alone so gather 0 can start ~200ns earlier; the
    # remaining chunks' idxs arrive well before their gathers issue.
    NIC = NI // 16
    sb_idx = pool.tile([P, NLOC * NIC], i16)
    nc.sync.dma_start(out=sb_idx[:, 0:NIC], in_=idx[:, 0:NIC])
    nc.sync.dma_start(out=sb_idx[:, NIC:], in_=idx[:, NIC:])
    sb_blob = pool.tile([P, BLOB], f32)
    nc.sync.dma_start(out=sb_blob, in_=blob)
    sb_oh = pool.tile([P, CELLS, ROW], bf16)
    nc.sync.dma_start(out=sb_oh, in_=oh)

    visp = sb_blob[:, C_VISP:C_VISP + K]
    pers30 = sb_blob[:, C_PERS:C_PERS + M]
    ind4 = sb_blob[:, C_IND:C_IND + SLOTS * MP]
    seg = sb_blob[:, C_SEG:C_SEG + NLOC]
    imgblk = sb_blob[:, C_IMG:C_IMG + P]

    # ---- per-image gathers (Pool SWDGE), issued ASAP ------------------------
    rows = []
    for c in range(NLOC):
        rows_c = pool.tile([P, SLOTS, ROW], f32, tag=f"rows{c}")
        nc.gpsimd.dma_gather(
            out_ap=rows_c[:],
            in_ap=tags_d[c],
            idxs_ap=sb_idx[:, c * (NI // 16):(c + 1) * (NI // 16)],
            num_idxs=NI,
            num_idxs_reg=NI,
            elem_size=ROW,
        )
        rows.append(rows_c)

    # ---- visibility-derived scalars (overlap the gathers) -------------------
    visfp = pool.tile([P, K], f32)         # person-layout 0/1 mask
    nc.vector.tensor_scalar(out=visfp, in0=visp, scalar1=0.0, scalar2=None,
                            op0=Alu.is_gt)
    cnt = pool.tile([P, 1], f32)
    nc.vector.reduce_sum(out=cnt, in_=visfp, axis=AxX)
    valid = pool.tile([P, 1], f32)
    nc.vector.tensor_scalar(out=valid, in0=cnt, scalar1=1.0, scalar2=None,
                            op0=Alu.min)
    safecnt = pool.tile([P, 1], f32)
    nc.vector.tensor_scalar(out=safecnt, in0=cnt, scalar1=1.0, scalar2=None,
                            op0=Alu.max)
    rc = pool.tile([P, 1], f32)
    nc.vector.reciprocal(out=rc, in_=safecnt)
    sm = pool.tile([P, 1], f32)            # BIGK*(1-valid)
    nc.vector.tensor_scalar(out=sm, in0=valid, scalar1=-BIG, scalar2=BIG,
                            op0=Alu.mult, op1=Alu.add)
    segv = pool.tile([P, NLOC], f32)       # SEG gated by person validity
    nc.vector.tensor_scalar(out=segv, in0=seg, scalar1=valid, scalar2=None,
                            op0=Alu.mult)

    # per-image tag counts + push denominators (all early, off critical path)
    psNT = psum.tile([NLOC, 1], f32, tag="psNT")
    nc.tensor.matmul(out=psNT[:], lhsT=seg, rhs=valid, start=True, stop=True)
    nt = pool.tile([NLOC, 1], f32)
    nc.vector.tensor_copy(out=nt, in_=psNT)
    sant = pool.tile([NLOC, 1], f32)
    nc.vector.tensor_scalar(out=sant, in0=nt, scalar1=1.0, scalar2=None,
                            op0=Alu.max)
    rnt = pool.tile([NLOC, 1], f32)
    nc.vector.reciprocal(out=rnt, in_=sant)
    npr = pool.tile([NLOC, 1], f32)
    nc.vector.scalar_tensor_tensor(out=npr, in0=nt, scalar=-1.0, in1=nt,
                                   op0=Alu.add, op1=Alu.mult)  # (nt-1)*nt
    gate = pool.tile([NLOC, 1], f32)
    nc.vector.tensor_scalar(out=gate, in0=npr, scalar1=0.0, scalar2=None,
                            op0=Alu.is_gt)
    sanp = pool.tile([NLOC, 1], f32)
    nc.vector.tensor_scalar(out=sanp, in0=npr, scalar1=1.0, scalar2=None,
                            op0=Alu.max)
    rnp = pool.tile([NLOC, 1], f32)
    nc.vector.reciprocal(out=rnp, in_=sanp)
    ga = pool.tile([NLOC, 1], f32)         # sqrt(pi)/4 * gate * rnp
    nc.vector.scalar_tensor_tensor(out=ga, in0=gate, scalar=SQPI_4, in1=rnp,
                                   op0=Alu.mult, op1=Alu.mult)
    gb0 = pool.tile([NLOC, 1], f32)
    nc.vector.scalar_tensor_tensor(out=gb0, in0=rnp, scalar=0.5, in1=nt,
                                   op0=Alu.mult, op1=Alu.mult)
    gb = pool.tile([NLOC, 1], f32)         # 0.5 * nt * gate * rnp
    nc.vector.tensor_tensor(out=gb, in0=gb0, in1=gate, op=Alu.mult)

    # ---- per-chunk selection + cell stats + person-stats matmul -------------
    GV = pool.tile([P, 2, CELLS], f32)
    fence = pool.tile([P, 1], f32)
    psS = psum.tile([P, 2], f32, tag="psS")
    sels = []
    for c in range(NLOC):
        sel_t = pool.tile([P, SLOTS, ROW], f32, tag=f"sel{c}", name=f"sel{c}")
        sels.append(sel_t)

    for c in range(NLOC):
        cs = slice(c * SLOTS, (c + 1) * SLOTS)
        sel_c = sels[c]
        nc.vector.tensor_tensor(out=sel_c, in0=rows[c][:],
                                in1=sb_oh[:, cs, :], op=Alu.mult)
        # onehot is visibility-gated, so the reduce gives sum v*g directly,
        # and since v in {0,1}: sum v*g^2 == sum (v*g)^2
        nc.vector.reduce_sum(out=GV[:, 0, cs], in_=sel_c[:], axis=AxX)
        nc.vector.tensor_tensor(out=GV[:, 1, cs], in0=GV[:, 0, cs],
                                in1=GV[:, 0, cs], op=Alu.mult)
        for s in range(SLOTS):
            nc.tensor.matmul(out=psS[c * MP:(c + 1) * MP, :],
                             lhsT=ind4[:, s * MP:(s + 1) * MP],
                             rhs=GV[:, :, c * SLOTS + s],
                             start=(s == 0), stop=(s == SLOTS - 1),
                             tile_position=(0, c * MP))
        if c + 1 < NLOC:
            # Scheduler fence: a throwaway [128,1] op that READS chunk c's
            # last gsv2 column and the next chunk's (not-yet-written) sel
            # tile.  The WAR hazard on sel_{c+1} forces the tile scheduler
            # to order sel_{c+1} after chunk c's stats on the DVE queue —
            # otherwise it hoists sel_{c+1} ahead and the last stats
            # matmul (which gates smm) lands ~0.5us late.
            nc.vector.tensor_tensor(
                out=fence, in0=GV[:, 1, c * SLOTS + SLOTS - 1:(c + 1) * SLOTS],
                in1=sels[c + 1][:, 0, 0:1], op=Alu.mult)

    # ---- per-person mean / push arg / pull ----------------------------------
    smm = pool.tile([P, 1], f32)           # mean + BIGK*(1-valid)
    nc.vector.scalar_tensor_tensor(out=smm, in0=psS[:, 0:1], scalar=rc,
                                   in1=sm, op0=Alu.mult, op1=Alu.add)
    mrhs = pool.tile([P, M], f32)
    nc.vector.tensor_scalar(out=mrhs, in0=pers30, scalar1=smm, scalar2=None,
                            op0=Alu.mult)
    psRep = psum.tile([P, M], f32, tag="psRep")
    nc.tensor.matmul(out=psRep[:], lhsT=imgblk, rhs=mrhs[:],
                     start=True, stop=True)
    negmean = pool.tile([P, 1], f32)
    nc.vector.tensor_scalar(out=negmean, in0=psS[:, 0:1], scalar1=rc,
                            scalar2=-1.0, op0=Alu.mult, op1=Alu.mult)
    mean2 = pool.tile([P, 1], f32)
    nc.vector.tensor_tensor(out=mean2, in0=negmean, in1=negmean, op=Alu.mult)
    p1 = pool.tile([P, 1], f32)
    nc.vector.scalar_tensor_tensor(out=p1, in0=psS[:, 1:2], scalar=rc,
                                   in1=mean2, op0=Alu.mult, op1=Alu.subtract)
    pullred = pool.tile([P, 1], f32)
    nc.vector.tensor_tensor(out=pullred, in0=p1, in1=valid, op=Alu.mult)

    # derf(arg) = (2/sqrt(pi)) exp(-arg^2); row sum in the ACT accumulator
    pe = pool.tile([P, M], f32)
    rowsumv = pool.tile([P, 1], f32)
    nc.scalar.activation(out=pe, in_=psRep[:], func=Act.Derivative_Erf,
                         bias=negmean, accum_out=rowsumv)

    # ---- per-image segment sums + finals ------------------------------------
    psF = psum.tile([NLOC, 2], f32, tag="psF")
    nc.tensor.matmul(out=psF[:, 0:1], lhsT=seg, rhs=pullred[:],
                     start=True, stop=True)
    nc.tensor.matmul(out=psF[:, 1:2], lhsT=segv, rhs=rowsumv[:],
                     start=True, stop=True)
    f42 = pool.tile([NLOC, 2], f32)
    nc.vector.tensor_scalar(out=f42[:, 0:1], in0=psF[:, 0:1], scalar1=rnt,
                            scalar2=None, op0=Alu.mult)
    nc.vector.scalar_tensor_tensor(out=f42[:, 1:2], in0=psF[:, 1:2],
                                   scalar=ga, in1=gb, op0=Alu.mult,
                                   op1=Alu.subtract)
    nc.sync.dma_start(out=out, in_=f42)


# ---------------------------------------------------------------------------
# host side
# ---------------------------------------------------------------------------

def _build_consts():
    p = np.arange(P)
    j30 = np.arange(M)
    pers30 = ((p[:, None] % MP) == j30[None, :]).astype(np.float32)
    # IND4[p, s*32+m] = 1 iff gather cell i = s*128+p is a real cell of
    # person m (column-major packing: cell i == m*K + k for i < M*K)
    i_grid = np.arange(SLOTS)[:, None] * P + p[None, :]      # [4, 128]
    ind4 = np.zeros((P, SLOTS, MP), dtype=np.float32)
    for s in range(SLOTS):
        mcell = i_grid[s] // K
        real = i_grid[s] < M * K
        ind4[real, s, mcell[real]] = 1.0
    seg = (((p[:, None] // MP) == np.arange(NLOC)[None, :])
           & ((p[:, None] % MP) < M)).astype(np.float32)
    imgblk = ((p[:, None] // MP) == (p[None, :] // MP)).astype(np.float32)
    return np.concatenate([pers30, ind4.reshape(P, SLOTS * MP), seg, imgblk],
                          axis=1).astype(np.float32)  # [128, 290]


_CONSTS = _build_consts()

# cell mapping: joint k of person m -> cell i = m*K + k; partition i%128,
# slot i//128 (matches the gather's idx->dest wrap)
_MM, _KK = np.meshgrid(np.arange(M), np.arange(K), indexing="ij")
_CI = (_MM * K + _KK).ravel()                       # [510]
_CELL_P = _CI % P
_CELL_S = _CI // P
_IDX_I = _CI                                        # gather idx position i


def make_in_maps(tags: np.ndarray, joints: np.ndarray):
    tags = np.ascontiguousarray(np.asarray(tags, dtype=np.float32))
    jt = np.asarray(joints)
    loc = np.clip(jt[..., 0], 0, KHW - 1).astype(np.int64)   # [N, M, K]
    visraw = jt[..., 1].astype(np.float32)                   # [N, M, K]
    row = (loc // ROW).astype(np.int16)                      # [N, M, K]
    sub = (loc % ROW).astype(np.int64)                       # [N, M, K]

    # gather idx arrays [N, 640] -> SBUF wrap [N, 128, 40]
    idx_all = np.zeros((N, NI), dtype=np.int16)
    idx_all[:, _IDX_I] = row.reshape(N, M * K)
    idx_sb = np.tile(idx_all.reshape(N, NI // 16, 16).transpose(0, 2, 1),
                     (1, 8, 1))                              # [N, 128, 40]

    # visibility-gated selection one-hot [N, 128, SLOTS*ROW]
    onehot = np.zeros((N, P, SLOTS * ROW), dtype=np.float32)
    cellflat = (_CELL_S * ROW)[None, :] + sub.reshape(N, M * K)  # [N, 510]
    onehot[np.arange(N)[:, None], _CELL_P[None, :], cellflat] = (
        visraw.reshape(N, M * K) > 0).astype(np.float32)

    # person-layout vis [N, 32, K]
    visp = np.zeros((N, MP, K), dtype=np.float32)
    visp[:, :M, :] = visraw

    in_maps = []
    for c in range(NCORES):
        sl = slice(c * NLOC, (c + 1) * NLOC)
        blob = np.concatenate([
            visp[sl].reshape(P, K),
            _CONSTS,
        ], axis=1)
        in_maps.append({
            "tags": tags[sl].reshape(NLOC, NROWS, ROW),
            "idx": np.ascontiguousarray(
                np.concatenate(idx_sb[sl], axis=1)),     # [128, 160]
            "blob": np.ascontiguousarray(blob),
            "oh": np.ascontiguousarray(
                onehot[sl].transpose(1, 0, 2).reshape(P, CELLS * ROW)
                .astype(ml_dtypes.bfloat16)),
        })
    return in_maps


_NC_CACHE = None


def _get_nc():
    global _NC_CACHE
    if _NC_CACHE is None:
        _NC_CACHE = build_nc()
    return _NC_CACHE


def kernel(tags: np.ndarray, joints: np.ndarray, _bench_results=None):
    nc = _get_nc()
    in_maps = make_in_maps(tags, joints)
    res = run_bass_kernel_spmd(nc, in_maps, core_ids=list(range(NCORES)))
    if _bench_results is not None:
        _bench_results.append(res)
    per_image = np.concatenate([r["out"] for r in res.results], axis=0)
    pull_loss = np.float32(per_image[:, 0].mean(dtype=np.float64))
    push_loss = np.float32(per_image[:, 1].mean(dtype=np.float64))
    return pull_loss, push_loss
